# revision 42
# baseline (speedup 1.0000x reference)
"""MedianBlur 3x3 raw-Bass v10: pair-shared vertical + fused E/O bands
+ cross-buffer fused single-band ops (measured ~119.0us vs 133us v5).

Notes:
- Vertical sort3 shares the adjacent-row min/max pair between the two
  windows that straddle it (windows 2t and 2t+1 both use pair
  (2t, 2t+1)): 5 ops/row of elements instead of 6, emitted as 2 pair
  ops on K/2 rows + 4 dual-parity window ops on K rows (a [3*RW, 2]
  AP dim for the singles, stride-0 broadcast for the pairs).
- Horizontal stage fuses E/O band op pairs into single 4D-AP ops
  (band dim stride 258/259, or stride 0 for shared operands), using
  min/max commutativity: 12 ops instead of 20 per pass.  All
  intermediates live in ONE super-buffer so same-ALU-op pairs from
  different logical buffers fuse too via a [delta, 2] AP dim
  ((mA,OPx) as one MAX, (OPn,mC) as one MIN): 16 ops/pass.
- 3 passes, 48 DVE ops total: K4(img0) / K16(imgs1-4) / K4(img5).
  All DMA packets are >= 6KB (no K2 passes): HWDGE queues move small
  packets at ~300-1000ns each, so packet count dominates the head.
- gpsimd's SWDGE queue is ~3x faster per packet than sync/scalar
  HWDGE, so it carries half of the P0 load; the 2.4MB K16 load is
  trigger-gated behind P0 (queues interleave packets of outstanding
  DMAs, which would starve the urgent pass-0 load).
- No dma_reset / start barrier (the NEFF framework preamble clears
  kernel semaphores itself); a defensive end-of-block sem_clear keeps
  repeat executions clean.
"""

import os

import numpy as np

import concourse.bacc as bacc
import concourse.bass as bass
import concourse.mybir as mybir
from concourse.bass_utils import run_bass_kernel_spmd

BF16 = mybir.dt.bfloat16
MIN = mybir.AluOpType.min
MAX = mybir.AluOpType.max

N_CORES = 8
B, C, H, W = 16, 3, 512, 512
IMGS = (B // N_CORES) * C  # 6
HP = H + 2
PW = 258          # plane width (257 valid + 1 pad)
RW = 2 * PW       # row stride [E|O] = 516
HALF = 256        # valid outputs per plane row

# (K rows/partition, img, rowbase); all passes use 128 partitions
PASSES = [(4, 0, 0), (16, 1, 0), (4, 5, 0)]
HH_OF = [0, 1, 2]
NP = len(PASSES)
LAST = NP - 1

_cache = {}


def _ap(handle, off, dims):
    """Free-dim AP over all 128 partitions of an sbuf tensor."""
    free = handle.shape[1]
    return bass.AP(handle, off, [[free, 128]] + dims)


# Sub-buffer offsets (elems) inside the single super-buffer SB.  Keeping
# every intermediate in one allocation lets ops fuse ACROSS logical
# buffers with a [delta, 2] AP dim (e.g. one MAX op computing both mA
# from L and OPx from M).
OFF_PN = 0
OFF_PX = 8 * RW
OFF_LB = 16 * RW
OFF_MB = 32 * RW
OFF_HH = [48 * RW, 52 * RW, 68 * RW]    # K4 / K16 / K4 passes
OFF_TA = 72 * RW          # mA (@E) / OPn (@O)
OFF_TC = 88 * RW          # mC (@E) / OPx (@O)
SB_ROWS = 104
# every cross-buffer AP delta must fit the 16-bit signed step_elem ISA
# field (<= 32767 elems); the largest here is MB->HH2 = 36*RW = 18576.


def _median_pass(V, Xf, SB, hh, K):
    """Vertical: 5 ops. Horizontal: 10 ops (cross-buffer fused singles)."""
    Kh = K // 2

    # --- vertical stage: shared-pair sort3 ---
    # tile row i = image row r0-1+i; out row i lives at buffer row i.
    # pair t = image rows (r0+2t, r0+2t+1) = tile rows (2t+1, 2t+2);
    # it serves window 2t (single = tile row 2t) and window 2t+1
    # (single = tile row 2t+3).
    pair_a = _ap(Xf, RW, [[2 * RW, Kh], [1, RW]])
    pair_b = _ap(Xf, 2 * RW, [[2 * RW, Kh], [1, RW]])
    pn3 = _ap(SB, OFF_PN, [[RW, Kh], [1, RW]])
    px3 = _ap(SB, OFF_PX, [[RW, Kh], [1, RW]])
    V.tensor_tensor(pn3, pair_a, pair_b, op=MIN)
    V.tensor_tensor(px3, pair_a, pair_b, op=MAX)

    S = _ap(Xf, 0, [[2 * RW, Kh], [3 * RW, 2], [1, RW]])
    pnb = _ap(SB, OFF_PN, [[RW, Kh], [0, 2], [1, RW]])
    pxb = _ap(SB, OFF_PX, [[RW, Kh], [0, 2], [1, RW]])
    # (a 4-free-dim fused (lo,t) op is illegal: the DVE ISA mem pattern
    # is capped at 3 free dims)
    Lw = _ap(SB, OFF_LB, [[2 * RW, Kh], [RW, 2], [1, RW]])
    Hw = _ap(SB, hh, [[2 * RW, Kh], [RW, 2], [1, RW]])
    Mw = _ap(SB, OFF_MB, [[2 * RW, Kh], [RW, 2], [1, RW]])
    V.tensor_tensor(Lw, S, pnb, op=MIN)   # lo  = min(s, Pn)
    V.tensor_tensor(Hw, S, pxb, op=MAX)   # hi  = max(s, Px)
    V.tensor_tensor(Mw, S, pxb, op=MIN)   # t   = min(s, Px)
    V.tensor_tensor(Mw, Mw, pnb, op=MAX)  # mid = max(t, Pn)

    # --- horizontal stage, E/O band pairs fused ---
    def b2x(off0, off1):     # band pair on the X scratch tile
        return _ap(Xf, off0, [[RW, K], [off1 - off0, 2], [1, HALF]])

    def b2(base, off0, off1):
        return _ap(SB, base + off0, [[RW, K], [off1 - off0, 2], [1, HALF]])

    def bs(off):             # shared operand broadcast over band dim
        return _ap(SB, off, [[RW, K], [0, 2], [1, HALF]])

    def du(base0, off0, base1, off1):  # cross-buffer pair (slot0, slot1)
        return _ap(
            SB, base0 + off0,
            [[(base1 + off1) - (base0 + off0), 2], [RW, K], [1, HALF]],
        )

    E, E1, O, O1 = 0, 1, PW, PW + 1
    # fused singles: (mA, OPx) = max((LO, MO), (LE1, ME1))
    V.tensor_tensor(
        du(OFF_TA, E, OFF_TC, O),
        du(OFF_LB, O, OFF_MB, O), du(OFF_LB, E1, OFF_MB, E1), op=MAX,
    )
    # fused singles: (OPn, mC) = min((MO, HO), (ME1, HE1))
    V.tensor_tensor(
        du(OFF_TA, O, OFF_TC, E),
        du(OFF_MB, O, hh, O), du(OFF_MB, E1, hh, E1), op=MIN,
    )
    # A = max3_h(L) -> X bands
    V.tensor_tensor(b2x(E, O), b2(OFF_LB, E, O1), bs(OFF_TA + E), op=MAX)
    # C = min3_h(H) -> L bands
    V.tensor_tensor(b2(OFF_LB, E, O), b2(hh, E, O1), bs(OFF_TC + E), op=MIN)
    # B = med3_h(M) -> H bands (shared middle pair OP = (MO, ME1))
    V.tensor_tensor(b2(hh, E, O), b2(OFF_MB, E, O1), bs(OFF_TC + O), op=MIN)
    V.tensor_tensor(b2(hh, E, O), b2(hh, E, O), bs(OFF_TA + O), op=MAX)
    # final med3(A=X, B=H, C=L) -> H bands
    V.tensor_tensor(b2(OFF_MB, E, O), b2x(E, O), b2(hh, E, O), op=MIN)   # U
    V.tensor_tensor(b2x(E, O), b2x(E, O), b2(hh, E, O), op=MAX)          # V
    V.tensor_tensor(b2x(E, O), b2x(E, O), b2(OFF_LB, E, O), op=MIN)      # W
    return V.tensor_tensor(b2(hh, E, O), b2(OFF_MB, E, O), b2x(E, O), op=MAX)


def _build():
    nc = bacc.Bacc(
        "TRN2", target_bir_lowering=False, debug=False, num_devices=N_CORES
    )
    xp = nc.declare_dram_parameter("xp", [IMGS, HP, RW], BF16, isOutput=False)
    y = nc.declare_dram_parameter("y", [IMGS, H, W], BF16, isOutput=True)

    Xs = [
        nc.alloc_sbuf_tensor(f"X{i}", [128, (K + 2) * RW], BF16)
        for i, (K, _, _) in enumerate(PASSES)
    ]
    SB = nc.alloc_sbuf_tensor("SB", [128, SB_ROWS * RW], BF16)

    def load_ap(ps, p0, npart):
        K, img, rowbase = PASSES[ps]
        pimg = H // K
        img = img + p0 // pimg
        row0 = rowbase + (p0 % pimg) * K
        return bass.AP(
            xp,
            img * HP * RW + row0 * RW,
            [[K * RW, npart], [1, (K + 2) * RW]],
        )

    def store_aps(ps, p0, npart):
        K, img, rowbase = PASSES[ps]
        pimg = H // K
        img = img + p0 // pimg
        row0 = rowbase + (p0 % pimg) * K
        dst = bass.AP(y, img * H * W + row0 * W, [[K * W, npart], [1, K * W]])
        hh = OFF_HH[HH_OF[ps]]
        src = SB[p0 : p0 + npart, hh : hh + K * RW].rearrange(
            "p (r b c) -> p r b c", b=2, c=PW
        )[:, 0:K, :, 0:HALF]
        return dst, src

    load_sems = [nc.alloc_semaphore(f"pload{i}") for i in range(NP)]
    dve_sem = nc.alloc_semaphore("pdve_sem")
    stB = nc.alloc_semaphore("pstB")

    nums = sorted(h.num for h in load_sems + [dve_sem, stB])
    lo, hi = nums[0], nums[-1]
    assert nums == list(range(lo, hi + 1)), nums
    sem_range = range(lo, hi + 1)
    # Semaphores are cleared at the END of the block (see blk.sync), so a
    # repeat execution starts clean without a start-of-kernel barrier.
    # First execution relies on NRT zero-initializing semaphores at load.

    # (pass, p0, npart) per trigger engine; each chunk incs its sem by 16.
    # Chunks never span an image boundary (DRAM rows are HP=514 per image,
    # so a linear [K*RW, npart] walk breaks at img edges).  The gpsimd
    # SWDGE queue moves small packets ~3x faster than the sync/scalar
    # HWDGE queues, so it gets a double share of the 6KB-packet P0 load.
    # Only the 2.4MB big-packet P1 load is gated (behind P0); P2 follows
    # ungated (needed only at ~95us).
    LOADS = {
        "sync": [(0, 77, 29), ("wait", 0), (1, 32, 32), ("wait", 1),
                 (2, 0, 64)],
        "scalar": [(0, 106, 22), ("wait", 0), (1, 64, 32), ("wait", 1),
                   (2, 64, 64)],
        "gpsimd": [(0, 0, 39), (0, 39, 38), ("wait", 0),
                   (1, 0, 32), (1, 96, 32)],
    }
    LOAD_THRESH = [64, 64, 32]
    STORES = {
        "sync": [(0, 0, 64), (1, 0, 32), (1, 64, 32), (2, 0, 32)],
        "scalar": [(0, 64, 64), (1, 32, 32), (1, 96, 32), (2, 32, 32)],
        "gpsimd": [(2, 64, 64)],
    }
    N_STB = 9           # total store chunks

    def emit_loads(eng, name):
        for entry in LOADS[name]:
            if entry[0] == "wait":
                eng.wait_ge(load_sems[entry[1]], LOAD_THRESH[entry[1]])
                continue
            ps, p0, npart = entry
            eng.dma_start(
                out=Xs[ps][p0 : p0 + npart, :], in_=load_ap(ps, p0, npart)
            ).then_inc(load_sems[ps], 16)

    def emit_stores(eng, name):
        cur = 0
        for ps, p0, npart in STORES[name]:
            if ps + 1 > cur:
                cur = ps + 1
                eng.wait_ge(dve_sem, cur)
            dst, src = store_aps(ps, p0, npart)
            eng.dma_start(out=dst, in_=src).then_inc(stB, 16)

    with nc.Block() as blk:

        @blk.sync
        def _(sync):
            emit_loads(sync, "sync")
            emit_stores(sync, "sync")
            sync.wait_ge(stB, N_STB * 16)
            sync.sem_clear(sem_range)  # clean state for a repeat execution

        @blk.scalar
        def _(scalar):
            emit_loads(scalar, "scalar")
            emit_stores(scalar, "scalar")

        @blk.gpsimd
        def _(gp):
            emit_loads(gp, "gpsimd")
            emit_stores(gp, "gpsimd")

        @blk.vector
        def _(V):
            for ps, (K, img, rowbase) in enumerate(PASSES):
                V.wait_ge(load_sems[ps], LOAD_THRESH[ps])
                _median_pass(
                    V, Xs[ps], SB, OFF_HH[HH_OF[ps]], K
                ).then_inc(dve_sem, 1)

    nc.finalize()
    return nc


LAST_EXEC_TIME_NS = None
LAST_TRACE = None


def _to_bf16_u16(a: np.ndarray) -> np.ndarray:
    u = a.view(np.uint32)
    r = ((u >> 16) & np.uint32(1)) + np.uint32(0x7FFF)
    return ((u + r) >> 16).astype(np.uint16)


def run(x: np.ndarray, trace: bool = False):
    global LAST_EXEC_TIME_NS, LAST_TRACE
    assert x.shape == (B, C, H, W), x.shape
    x = np.ascontiguousarray(x, dtype=np.float32)

    import ml_dtypes

    if "P" not in _cache:
        _cache["P"] = _build()
    nc = _cache["P"]

    xpad = np.pad(x, ((0, 0), (0, 0), (1, 1), (1, 1)))  # (B,C,514,514)
    planes = np.zeros((B, C, HP, 2, PW), dtype=np.float32)
    planes[..., 0, :257] = xpad[..., 0::2]
    planes[..., 1, :257] = xpad[..., 1::2]
    xb = _to_bf16_u16(np.ascontiguousarray(planes)).view(ml_dtypes.bfloat16)
    shards = xb.reshape(N_CORES, IMGS, HP, RW)
    in_maps = [{"xp": shards[c]} for c in range(N_CORES)]

    if not trace:
        os.environ["BASS_NEVER_TRACE"] = "1"
    else:
        os.environ.pop("BASS_NEVER_TRACE", None)
    res = run_bass_kernel_spmd(nc, in_maps, list(range(N_CORES)), trace=trace)
    LAST_EXEC_TIME_NS = res.exec_time_ns
    LAST_TRACE = res.instructions_and_trace
    yp = np.stack(
        [np.asarray(res.results[c]["y"]).astype(np.float32) for c in range(N_CORES)]
    ).reshape(B, C, H, 2, HALF)
    out = np.empty((B, C, H, W), dtype=np.float32)
    out[..., 0::2] = yp[..., 0, :]
    out[..., 1::2] = yp[..., 1, :]
    return out


def kernel(x: np.ndarray) -> np.ndarray:
    return run(x, trace=False)


# revision 43
# speedup vs baseline: 1.0444x; 1.0444x over previous
"""MedianBlur 3x3 raw-Bass v10: pair-shared vertical + fused E/O bands
+ cross-buffer fused single-band ops (measured ~119.0us vs 133us v5).

Notes:
- Vertical sort3 shares the adjacent-row min/max pair between the two
  windows that straddle it (windows 2t and 2t+1 both use pair
  (2t, 2t+1)): 5 ops/row of elements instead of 6, emitted as 2 pair
  ops on K/2 rows + 4 dual-parity window ops on K rows (a [3*RW, 2]
  AP dim for the singles, stride-0 broadcast for the pairs).
- Horizontal stage fuses E/O band op pairs into single 4D-AP ops
  (band dim stride 258/259, or stride 0 for shared operands), using
  min/max commutativity: 12 ops instead of 20 per pass.  All
  intermediates live in ONE super-buffer so same-ALU-op pairs from
  different logical buffers fuse too via a [delta, 2] AP dim
  ((mA,OPx) as one MAX, (OPn,mC) as one MIN): 16 ops/pass.
- 3 passes, 48 DVE ops total: K4(img0) / K16(imgs1-4) / K4(img5).
  All DMA packets are >= 6KB (no K2 passes): HWDGE queues move small
  packets at ~300-1000ns each, so packet count dominates the head.
- gpsimd's SWDGE queue is ~3x faster per packet than sync/scalar
  HWDGE, so it carries half of the P0 load; the 2.4MB K16 load is
  trigger-gated behind P0 (queues interleave packets of outstanding
  DMAs, which would starve the urgent pass-0 load).
- No dma_reset / start barrier (the NEFF framework preamble clears
  kernel semaphores itself); a defensive end-of-block sem_clear keeps
  repeat executions clean.
"""

import os

import numpy as np

import concourse.bacc as bacc
import concourse.bass as bass
import concourse.mybir as mybir
from concourse.bass_utils import run_bass_kernel_spmd

BF16 = mybir.dt.bfloat16
MIN = mybir.AluOpType.min
MAX = mybir.AluOpType.max

N_CORES = 8
B, C, H, W = 16, 3, 512, 512
IMGS = (B // N_CORES) * C  # 6
HP = H + 2
PW = 258          # plane width (257 valid + 1 pad)
RW = 2 * PW       # row stride [E|O] = 516
HALF = 256        # valid outputs per plane row

# (K rows/partition, img, rowbase); all passes use 128 partitions
PASSES = [(4, 0, 0), (16, 1, 0), (4, 5, 0)]
HH_OF = [0, 1, 2]
NP = len(PASSES)
LAST = NP - 1

_cache = {}


def _ap(handle, off, dims):
    """Free-dim AP over all 128 partitions of an sbuf tensor."""
    free = handle.shape[1]
    return bass.AP(handle, off, [[free, 128]] + dims)


# Sub-buffer offsets (elems) inside the single super-buffer SB.  Keeping
# every intermediate in one allocation lets ops fuse ACROSS logical
# buffers with a [delta, 2] AP dim (e.g. one MAX op computing both mA
# from L and OPx from M).
OFF_PN = 0
OFF_PX = 8 * RW
OFF_LB = 16 * RW
OFF_MB = 32 * RW
OFF_HH = [48 * RW, 52 * RW, 68 * RW]    # K4 / K16 / K4 passes
OFF_TA = 72 * RW          # mA (@E) / OPn (@O)
OFF_TC = 88 * RW          # mC (@E) / OPx (@O)
SB_ROWS = 104
# every cross-buffer AP delta must fit the 16-bit signed step_elem ISA
# field (<= 32767 elems); the largest here is MB->HH2 = 36*RW = 18576.


def _median_pass(V, Xf, SB, hh, K):
    """Vertical: 5 ops. Horizontal: 10 ops (cross-buffer fused singles)."""
    Kh = K // 2

    # --- vertical stage: shared-pair sort3 ---
    # tile row i = image row r0-1+i; out row i lives at buffer row i.
    # pair t = image rows (r0+2t, r0+2t+1) = tile rows (2t+1, 2t+2);
    # it serves window 2t (single = tile row 2t) and window 2t+1
    # (single = tile row 2t+3).
    pair_a = _ap(Xf, RW, [[2 * RW, Kh], [1, RW]])
    pair_b = _ap(Xf, 2 * RW, [[2 * RW, Kh], [1, RW]])
    pn3 = _ap(SB, OFF_PN, [[RW, Kh], [1, RW]])
    px3 = _ap(SB, OFF_PX, [[RW, Kh], [1, RW]])
    V.tensor_tensor(pn3, pair_a, pair_b, op=MIN)
    V.tensor_tensor(px3, pair_a, pair_b, op=MAX)

    S = _ap(Xf, 0, [[2 * RW, Kh], [3 * RW, 2], [1, RW]])
    pnb = _ap(SB, OFF_PN, [[RW, Kh], [0, 2], [1, RW]])
    pxb = _ap(SB, OFF_PX, [[RW, Kh], [0, 2], [1, RW]])
    # (a 4-free-dim fused (lo,t) op is illegal: the DVE ISA mem pattern
    # is capped at 3 free dims)
    Lw = _ap(SB, OFF_LB, [[2 * RW, Kh], [RW, 2], [1, RW]])
    Hw = _ap(SB, hh, [[2 * RW, Kh], [RW, 2], [1, RW]])
    Mw = _ap(SB, OFF_MB, [[2 * RW, Kh], [RW, 2], [1, RW]])
    V.tensor_tensor(Lw, S, pnb, op=MIN)   # lo  = min(s, Pn)
    V.tensor_tensor(Hw, S, pxb, op=MAX)   # hi  = max(s, Px)
    V.tensor_tensor(Mw, S, pxb, op=MIN)   # t   = min(s, Px)
    V.tensor_tensor(Mw, Mw, pnb, op=MAX)  # mid = max(t, Pn)

    # --- horizontal stage, E/O band pairs fused ---
    def b2x(off0, off1):     # band pair on the X scratch tile
        return _ap(Xf, off0, [[RW, K], [off1 - off0, 2], [1, HALF]])

    def b2(base, off0, off1):
        return _ap(SB, base + off0, [[RW, K], [off1 - off0, 2], [1, HALF]])

    def bs(off):             # shared operand broadcast over band dim
        return _ap(SB, off, [[RW, K], [0, 2], [1, HALF]])

    def du(base0, off0, base1, off1):  # cross-buffer pair (slot0, slot1)
        return _ap(
            SB, base0 + off0,
            [[(base1 + off1) - (base0 + off0), 2], [RW, K], [1, HALF]],
        )

    E, E1, O, O1 = 0, 1, PW, PW + 1
    # fused singles: (mA, OPx) = max((LO, MO), (LE1, ME1))
    V.tensor_tensor(
        du(OFF_TA, E, OFF_TC, O),
        du(OFF_LB, O, OFF_MB, O), du(OFF_LB, E1, OFF_MB, E1), op=MAX,
    )
    # fused singles: (OPn, mC) = min((MO, HO), (ME1, HE1))
    V.tensor_tensor(
        du(OFF_TA, O, OFF_TC, E),
        du(OFF_MB, O, hh, O), du(OFF_MB, E1, hh, E1), op=MIN,
    )
    # A = max3_h(L) -> X bands
    V.tensor_tensor(b2x(E, O), b2(OFF_LB, E, O1), bs(OFF_TA + E), op=MAX)
    # C = min3_h(H) -> L bands
    V.tensor_tensor(b2(OFF_LB, E, O), b2(hh, E, O1), bs(OFF_TC + E), op=MIN)
    # B = med3_h(M) -> H bands (shared middle pair OP = (MO, ME1))
    V.tensor_tensor(b2(hh, E, O), b2(OFF_MB, E, O1), bs(OFF_TC + O), op=MIN)
    V.tensor_tensor(b2(hh, E, O), b2(hh, E, O), bs(OFF_TA + O), op=MAX)
    # final med3(A=X, B=H, C=L) -> H bands
    V.tensor_tensor(b2(OFF_MB, E, O), b2x(E, O), b2(hh, E, O), op=MIN)   # U
    V.tensor_tensor(b2x(E, O), b2x(E, O), b2(hh, E, O), op=MAX)          # V
    V.tensor_tensor(b2x(E, O), b2x(E, O), b2(OFF_LB, E, O), op=MIN)      # W
    return V.tensor_tensor(b2(hh, E, O), b2(OFF_MB, E, O), b2x(E, O), op=MAX)


def _build():
    nc = bacc.Bacc(
        "TRN2", target_bir_lowering=False, debug=False, num_devices=N_CORES
    )
    xp = nc.declare_dram_parameter("xp", [IMGS, HP, RW], BF16, isOutput=False)
    y = nc.declare_dram_parameter("y", [IMGS, H, W], BF16, isOutput=True)

    Xs = [
        nc.alloc_sbuf_tensor(f"X{i}", [128, (K + 2) * RW], BF16)
        for i, (K, _, _) in enumerate(PASSES)
    ]
    SB = nc.alloc_sbuf_tensor("SB", [128, SB_ROWS * RW], BF16)

    def load_ap(ps, p0, npart):
        K, img, rowbase = PASSES[ps]
        pimg = H // K
        img = img + p0 // pimg
        row0 = rowbase + (p0 % pimg) * K
        return bass.AP(
            xp,
            img * HP * RW + row0 * RW,
            [[K * RW, npart], [1, (K + 2) * RW]],
        )

    def store_aps(ps, p0, npart):
        K, img, rowbase = PASSES[ps]
        pimg = H // K
        img = img + p0 // pimg
        row0 = rowbase + (p0 % pimg) * K
        dst = bass.AP(y, img * H * W + row0 * W, [[K * W, npart], [1, K * W]])
        hh = OFF_HH[HH_OF[ps]]
        src = SB[p0 : p0 + npart, hh : hh + K * RW].rearrange(
            "p (r b c) -> p r b c", b=2, c=PW
        )[:, 0:K, :, 0:HALF]
        return dst, src

    load_sems = [nc.alloc_semaphore(f"pload{i}") for i in range(NP)]
    dve_sem = nc.alloc_semaphore("pdve_sem")
    stB = nc.alloc_semaphore("pstB")

    nums = sorted(h.num for h in load_sems + [dve_sem, stB])
    lo, hi = nums[0], nums[-1]
    assert nums == list(range(lo, hi + 1)), nums
    sem_range = range(lo, hi + 1)
    # Semaphores are cleared at the END of the block (see blk.sync), so a
    # repeat execution starts clean without a start-of-kernel barrier.
    # First execution relies on NRT zero-initializing semaphores at load.

    # (pass, p0, npart) per trigger engine; each chunk incs its sem by 16.
    # Chunks never span an image boundary (DRAM rows are HP=514 per image,
    # so a linear [K*RW, npart] walk breaks at img edges).  The gpsimd
    # SWDGE queue moves small packets ~3x faster than the sync/scalar
    # HWDGE queues, so it gets a double share of the 6KB-packet P0 load.
    # Only the 2.4MB big-packet P1 load is gated (behind P0); P2 follows
    # ungated (needed only at ~95us).
    LOADS = {
        "sync": [(0, 64, 32), ("wait", 0), (1, 32, 32), ("wait", 1),
                 (2, 0, 64)],
        "scalar": [(0, 96, 32), ("wait", 0), (1, 64, 32), ("wait", 1),
                   (2, 64, 64)],
        "gpsimd": [(0, 0, 32), (0, 32, 32), ("wait", 0),
                   (1, 0, 32), (1, 96, 32)],
    }
    LOAD_THRESH = [64, 64, 32]
    STORES = {
        "sync": [(0, 0, 64), (1, 0, 32), (1, 64, 32), (2, 0, 32)],
        "scalar": [(0, 64, 64), (1, 32, 32), (1, 96, 32), (2, 32, 32)],
        "gpsimd": [(2, 64, 64)],
    }
    N_STB = 9           # total store chunks

    def emit_loads(eng, name):
        for entry in LOADS[name]:
            if entry[0] == "wait":
                eng.wait_ge(load_sems[entry[1]], LOAD_THRESH[entry[1]])
                continue
            ps, p0, npart = entry
            eng.dma_start(
                out=Xs[ps][p0 : p0 + npart, :], in_=load_ap(ps, p0, npart)
            ).then_inc(load_sems[ps], 16)

    def emit_stores(eng, name):
        cur = 0
        for ps, p0, npart in STORES[name]:
            if ps + 1 > cur:
                cur = ps + 1
                eng.wait_ge(dve_sem, cur)
            dst, src = store_aps(ps, p0, npart)
            eng.dma_start(out=dst, in_=src).then_inc(stB, 16)

    with nc.Block() as blk:

        @blk.sync
        def _(sync):
            emit_loads(sync, "sync")
            emit_stores(sync, "sync")
            sync.wait_ge(stB, N_STB * 16)
            sync.sem_clear(sem_range)  # clean state for a repeat execution

        @blk.scalar
        def _(scalar):
            emit_loads(scalar, "scalar")
            emit_stores(scalar, "scalar")

        @blk.gpsimd
        def _(gp):
            emit_loads(gp, "gpsimd")
            emit_stores(gp, "gpsimd")

        @blk.vector
        def _(V):
            for ps, (K, img, rowbase) in enumerate(PASSES):
                V.wait_ge(load_sems[ps], LOAD_THRESH[ps])
                _median_pass(
                    V, Xs[ps], SB, OFF_HH[HH_OF[ps]], K
                ).then_inc(dve_sem, 1)

    nc.finalize()
    return nc


LAST_EXEC_TIME_NS = None
LAST_TRACE = None


def _to_bf16_u16(a: np.ndarray) -> np.ndarray:
    u = a.view(np.uint32)
    r = ((u >> 16) & np.uint32(1)) + np.uint32(0x7FFF)
    return ((u + r) >> 16).astype(np.uint16)


def run(x: np.ndarray, trace: bool = False):
    global LAST_EXEC_TIME_NS, LAST_TRACE
    assert x.shape == (B, C, H, W), x.shape
    x = np.ascontiguousarray(x, dtype=np.float32)

    import ml_dtypes

    if "P" not in _cache:
        _cache["P"] = _build()
    nc = _cache["P"]

    xpad = np.pad(x, ((0, 0), (0, 0), (1, 1), (1, 1)))  # (B,C,514,514)
    planes = np.zeros((B, C, HP, 2, PW), dtype=np.float32)
    planes[..., 0, :257] = xpad[..., 0::2]
    planes[..., 1, :257] = xpad[..., 1::2]
    xb = _to_bf16_u16(np.ascontiguousarray(planes)).view(ml_dtypes.bfloat16)
    shards = xb.reshape(N_CORES, IMGS, HP, RW)
    in_maps = [{"xp": shards[c]} for c in range(N_CORES)]

    if not trace:
        os.environ["BASS_NEVER_TRACE"] = "1"
    else:
        os.environ.pop("BASS_NEVER_TRACE", None)
    res = run_bass_kernel_spmd(nc, in_maps, list(range(N_CORES)), trace=trace)
    LAST_EXEC_TIME_NS = res.exec_time_ns
    LAST_TRACE = res.instructions_and_trace
    yp = np.stack(
        [np.asarray(res.results[c]["y"]).astype(np.float32) for c in range(N_CORES)]
    ).reshape(B, C, H, 2, HALF)
    out = np.empty((B, C, H, W), dtype=np.float32)
    out[..., 0::2] = yp[..., 0, :]
    out[..., 1::2] = yp[..., 1, :]
    return out


def kernel(x: np.ndarray) -> np.ndarray:
    return run(x, trace=False)
